# revision 54
# baseline (speedup 1.0000x reference)
"""SGC (2-hop simple graph convolution) Trainium2 kernel, 8-core SPMD.

out = S S x W^T + b,  S = D^{-1/2} (A + I) D^{-1/2}   (D = in-degree + 1)

Strategy:
  * project first: y = x @ W^T (64 ch), exact by associativity
  * factor norms:  S z = dinv * (A+I) (dinv * z)  -> per-node scalings only,
    messages are unweighted; self loop handled as a local add
  * per core: own 1/8 of destination nodes; edges partitioned by dst
  * gather sources with gpsimd dma_gather from an AllGather'ed bf16 table
    (rows padded to 128 ch = 256 B to satisfy elem%256 and int16 idx needs
    the table split in two 32768-row halves -> two message streams A/B)
  * scatter-adds via PE matmul: 128-message tiles x host-built 0/1 one-hot
    stationary tiles (shipped fp8, DMA-cast to bf16), accumulated in PSUM
    per 128-destination window; out-of-window slots give all-zero rows so
    stream tiles may straddle windows with no padding
  * node numbering inside tables is permuted (n -> p*WIN+w) so SBUF staging
    [128p, WIN, ch] maps contiguously to DRAM; host un-permutes at the end
"""

import sys

sys.path.insert(0, "/opt/trn_rl_repo")

import numpy as np
import ml_dtypes

# ---------------- problem constants (overridden by tests for small runs) ----
CFG = dict(
    N_NODES=65536,
    N_EDGES=655360,
    IN_C=128,
    OUT_C=64,
    CORES=8,
    CH=32,  # gather tiles (128 msgs each) per dma_gather call
    CHP=32,  # one-hot pairs per DMA chunk
    OC_PAD=128,  # bf16 channels per gather-table row (256 B)
    MSG_BUFS=5,
    OH_BUFS=4,
    PSUM_BUFS=6,
    RESYNC_G=16,
    ACT_EVAC=1,
    STAGE=6,
    SP=True,  # single_packet on gathers (safe only for num_idxs <= 1024)  # debug: 1 proj, 2 +ag1, 3 +gather/oh, 4 +hop1 mm, 5 +ag2, 6 full
    NQ=4,  # SWDGE queues; gathers round-robin across queue core-pairs
    OH_SYNC=1,  # one-hot tile loads on HWDGE (sync) instead of pool
)

SENT = 1 << 20  # sentinel "dst" for pad rows -> all-zero one-hot everywhere

FP8_ONE = 0x38  # float8_e4m3 bit pattern of 1.0


class Prep:
    pass


def _preprocess(edge_index):
    N = CFG["N_NODES"]
    C = CFG["CORES"]
    NP = N // C
    WIN = NP // 128
    WH = WIN // 2
    HALF = N // 2

    src = np.asarray(edge_index[0], dtype=np.int64)
    dst = np.asarray(edge_index[1], dtype=np.int64)
    deg = np.bincount(dst, minlength=N).astype(np.float32) + 1.0

    # stream split by source WINDOW half so each half's table can be
    # AllGather'ed as soon as that half of the windows is evacuated.
    # table row within stream: core*(NP/2) + p*WH + (w % WH)  (< 32768, int16)
    n_all = np.arange(N, dtype=np.int64)
    c_ = n_all // NP
    r_ = n_all % NP
    p_ = r_ % 128
    w_ = r_ // 128
    stream_of = (w_ >= WH).astype(np.int64)
    srow_of = c_ * (NP // 2) + p_ * WH + (w_ % WH)

    pr = Prep()
    pr.N, pr.C, pr.NP, pr.WIN, pr.HALF = N, C, NP, WIN, HALF
    pr.WH = WH

    # per-core, per-stream sorted message lists
    core_ld = [[None, None] for _ in range(C)]  # local dst per stream
    core_idx = [[None, None] for _ in range(C)]  # table idx per stream
    for i in range(C):
        m = (dst >= i * NP) & (dst < (i + 1) * NP)
        s_i = src[m]
        ld_i = dst[m] - i * NP
        order = np.argsort(ld_i, kind="stable")
        s_i, ld_i = s_i[order], ld_i[order]
        st_i = stream_of[s_i]
        rows = srow_of[s_i]
        for s in range(2):
            a = st_i == s
            core_ld[i][s], core_idx[i][s] = ld_i[a], rows[a]

    # re-align all cores' streams at every RESYNC_G windows: within a group,
    # pad each core's segment to the max core's tile count, so tile t sits in
    # the same window neighborhood on every core (cuts union-pair straddle).
    G = CFG.get("RESYNC_G", 16)
    n_groups = (WIN + G - 1) // G
    for s in range(2):
        seg_tiles = np.zeros(n_groups, dtype=np.int64)
        for g in range(n_groups):
            lo, hi = g * G * 128, min((g + 1) * G, WIN) * 128
            for i in range(C):
                cnt = int(((core_ld[i][s] >= lo) & (core_ld[i][s] < hi)).sum())
                seg_tiles[g] = max(seg_tiles[g], (cnt + 127) // 128)
        for i in range(C):
            lds, ixs = [], []
            for g in range(n_groups):
                lo, hi = g * G * 128, min((g + 1) * G, WIN) * 128
                m = (core_ld[i][s] >= lo) & (core_ld[i][s] < hi)
                ld_g, ix_g = core_ld[i][s][m], core_idx[i][s][m]
                pad = int(seg_tiles[g]) * 128 - len(ld_g)
                lds.append(np.concatenate([ld_g, np.full(pad, SENT, np.int64)]))
                ixs.append(np.concatenate([ix_g, np.zeros(pad, np.int64)]))
            core_ld[i][s] = np.concatenate(lds)
            core_idx[i][s] = np.concatenate(ixs)
    T = [len(core_ld[0][0]) // 128, len(core_ld[0][1]) // 128]
    pr.T = T

    for i in range(C):
        for s in range(2):
            assert len(core_ld[i][s]) == T[s] * 128

    # union pair structure (w, stream, tile) across cores; pairs ordered
    # stream-major so each hop runs as two passes (all A, then all B) with
    # every window's psum kept resident across the passes
    pair_set = set()
    for i in range(C):
        for s in range(2):
            L = core_ld[i][s].reshape(T[s], 128)
            for t in range(T[s]):
                real = L[t][L[t] != SENT]
                if len(real) == 0:
                    continue
                for w in range(int(real.min()) // 128, int(real.max()) // 128 + 1):
                    pair_set.add((w, s, t))
    for w in range(WIN):  # every window needs >=1 pair so psum gets reset
        if not any(p[0] == w for p in pair_set):
            pair_set.add((w, 0, 0))
    pairs = sorted(pair_set, key=lambda p: (p[1], p[0], p[2]))
    pr.pairs = pairs
    pr.n_pairs = len(pairs)
    segs = [[[], []] for _ in range(WIN)]
    for k, (w, s, t) in enumerate(pairs):
        segs[w][s].append(k)
    pr.segs = segs

    # per-core one-hot tiles [128, n_pairs, 128] bf16(0/1)
    pr.onehot = []
    pr.idx_wrapped = []
    pr.deg_staged = []
    for i in range(C):
        oh = np.zeros((128, pr.n_pairs, 128), dtype=np.uint16)
        for k, (w, s, t) in enumerate(pairs):
            ld_t = core_ld[i][s][t * 128 : (t + 1) * 128]
            slot = ld_t - 128 * w
            valid = (slot >= 0) & (slot < 128)
            rr = np.nonzero(valid)[0]
            oh[rr, k, slot[rr]] = 0x3F80  # bf16 1.0
        pr.onehot.append(oh.view(ml_dtypes.bfloat16))

        blocks = []
        for s in range(2):
            ix = core_idx[i][s].astype(np.int16)
            assert (core_idx[i][s] < 32768).all() and (core_idx[i][s] >= 0).all()
            w16 = ix.reshape(-1, 16).T  # [16, T*8]
            blocks.append(np.tile(w16, (8, 1)))  # replicate to 128 partitions
        pr.idx_wrapped.append(
            np.ascontiguousarray(np.concatenate(blocks, axis=1))
        )

        dshard = deg[i * NP : (i + 1) * NP]
        pr.deg_staged.append(
            np.ascontiguousarray(dshard.reshape(WIN, 128).T.astype(np.float32))
        )

    return pr


# ------------------------------------------------------------------ bass ----


def _build(pr):
    import concourse.bass as bass
    import concourse.bacc as bacc
    import concourse.mybir as mybir
    import concourse.tile as tile
    from concourse._compat import get_trn_type

    dt = mybir.dt
    Alu = mybir.AluOpType
    F32, BF16, FP8, I16 = dt.float32, dt.bfloat16, dt.float8e4, dt.int16

    IN_C, OUT_C = CFG["IN_C"], CFG["OUT_C"]
    OC_PAD, CH, CHP = CFG["OC_PAD"], CFG["CH"], CFG["CHP"]
    N, C, NP, WIN, HALF = pr.N, pr.C, pr.NP, pr.WIN, pr.HALF
    WH = pr.WH
    T = pr.T

    nc = bacc.Bacc(
        get_trn_type() or "TRN2",
        target_bir_lowering=False,
        debug=False,
        num_devices=C,
        num_swdge_queues=CFG["NQ"],
    )

    xt_d = nc.dram_tensor("xt", [IN_C, NP], F32, kind="ExternalInput")
    wt_d = nc.dram_tensor("wt", [IN_C, OUT_C], F32, kind="ExternalInput")
    b_d = nc.dram_tensor("bias", [128, OUT_C], F32, kind="ExternalInput")
    deg_d = nc.dram_tensor("deg", [128, WIN], F32, kind="ExternalInput")
    idx_d = nc.dram_tensor(
        "idx", [128, (T[0] + T[1]) * 8], I16, kind="ExternalInput"
    )
    oh_d = nc.dram_tensor("oh", [128, pr.n_pairs, 128], BF16, kind="ExternalInput")
    out_d = nc.dram_tensor("out", [NP, OUT_C], F32, kind="ExternalOutput")

    rg = [list(range(C))]

    with tile.TileContext(nc) as tc:
        with (
            tc.tile_pool(name="const", bufs=1) as const,
            tc.tile_pool(name="dram", bufs=1, space="DRAM") as dram,
            tc.tile_pool(name="psum", bufs=8, space="PSUM") as psum_pool,
            tc.tile_pool(name="msgA", bufs=CFG["MSG_BUFS"]) as msgA_pool,
            tc.tile_pool(name="msgB", bufs=CFG["MSG_BUFS"]) as msgB_pool,
            tc.tile_pool(name="ohp", bufs=CFG["OH_BUFS"]) as oh_pool,
            tc.tile_pool(name="tmp", bufs=4) as tmp_pool,
        ):
            cc_in = [
                [dram.tile([NP // 2, OC_PAD], BF16, name=f"cci{h}{s}") for s in range(2)]
                for h in range(2)
            ]
            cc_out = [
                [
                    dram.tile(
                        [N // 2, OC_PAD], BF16, addr_space="Shared", name=f"cco{h}{s}"
                    )
                    for s in range(2)
                ]
                for h in range(2)
            ]

            idx_sb = const.tile([128, (T[0] + T[1]) * 8], I16)
            nc.sync.dma_start(idx_sb[:], idx_d[:])
            wt_sb = const.tile([IN_C, OUT_C], F32)
            nc.sync.dma_start(wt_sb[:], wt_d[:])
            b_sb = const.tile([128, OUT_C], F32)
            nc.sync.dma_start(b_sb[:], b_d[:])
            deg_sb = const.tile([128, WIN], F32)
            nc.sync.dma_start(deg_sb[:], deg_d[:])
            deginv = const.tile([128, WIN], F32)
            nc.vector.reciprocal(deginv[:], deg_sb[:])
            dinv = const.tile([128, WIN], F32)
            nc.scalar.activation(
                dinv[:], deginv[:], mybir.ActivationFunctionType.Sqrt
            )

            outst = const.tile([128, WIN, OUT_C], F32)
            zpad1 = const.tile([128, WIN, OC_PAD], BF16)
            zpad2 = const.tile([128, WIN, OC_PAD], BF16)
            nc.vector.memset(zpad1[:], 0.0)
            nc.vector.memset(zpad2[:], 0.0)

            def emit_ag(h, s, zpad):
                # stage stream-s half of zpad into DRAM (scalar HWDGE, so it
                # queues right behind that half's evac writes) and AllGather it
                lo = s * WH
                nc.scalar.dma_start(cc_in[h][s][:], zpad[:, lo : lo + WH, :])
                nc.gpsimd.collective_compute(
                    "AllGather",
                    Alu.bypass,
                    replica_groups=rg,
                    ins=[cc_in[h][s].opt()],
                    outs=[cc_out[h][s].opt()],
                )

            # ---- projection: z0 = dinv * (x @ W^T), staged [p, w, ch] ----
            XCH = 16  # windows per streamed xt chunk
            xt_tiles = {}
            for r in range(WIN):
                if r % XCH == 0:
                    xc = tmp_pool.tile([IN_C, XCH * 128], F32, tag="xt", bufs=2)
                    nc.sync.dma_start(
                        xc[:], xt_d[:, r * 128 : (r + XCH) * 128]
                    )
                    xt_tiles[r // XCH] = xc
                xt_sb = xt_tiles[r // XCH]
                if r % 8 == 0:
                    pyb = psum_pool.tile([128, 8, OUT_C], F32, tag="w")
                py = pyb[:, r % 8, :]
                # one accumulation group per PSUM bank: start zeroes the whole
                # 2 KB zero-region, so only the bank's first matmul may start
                nc.tensor.matmul(
                    py,
                    xt_sb[:, (r % XCH) * 128 : (r % XCH + 1) * 128],
                    wt_sb[:],
                    start=(r % 8 == 0),
                    stop=(r % 8 == 7),
                )
                if CFG["ACT_EVAC"]:
                    nc.scalar.mul(zpad1[:, r, 0:OUT_C], py, dinv[:, r : r + 1])
                else:
                    nc.vector.tensor_scalar(
                        zpad1[:, r, 0:OUT_C], py, dinv[:, r : r + 1], None, Alu.mult
                    )
                if r == WH - 1:
                    emit_ag(0, 0, zpad1)
                elif r == WIN - 1:
                    emit_ag(0, 1, zpad1)

            # variable gather-call sizes: small chunks at each stream's head
            # so the first windows' messages land sooner after the AllGather
            def mk_plan(Tn):
                plan, rem = [], Tn
                for sz in (8, 8, 16):
                    if rem <= 0:
                        break
                    take = min(sz, rem)
                    plan.append(take)
                    rem -= take
                while rem > 0:
                    take = min(CH, rem)
                    plan.append(take)
                    rem -= take
                return plan

            plans = [mk_plan(T[0]), mk_plan(T[1])]
            offs, call_of, slot_of = [], [], []
            for s in range(2):
                off, co, so, acc = [], [], [], 0
                for c, n in enumerate(plans[s]):
                    off.append(acc)
                    co += [c] * n
                    so += list(range(n))
                    acc += n
                offs.append(off)
                call_of.append(co)
                slot_of.append(so)
            calls = [len(plans[0]), len(plans[1])]
            n_oh_chunks = (pr.n_pairs + CHP - 1) // CHP
            colbase = [0, T[0] * 8]

            qctr = [0]

            def run_hop(tabs, evac, mid_cb=None, end_cb=None):
                pools = [msgA_pool, msgB_pool]
                msg_tiles = [{}, {}]
                oh_tiles = {}
                next_call = [0, 0]
                next_oh = [0]

                def emit_gather(s):
                    c = next_call[s]
                    ntiles = plans[s][c]
                    off = offs[s][c]
                    ni = ntiles * 128
                    t = pools[s].tile([128, CH, OC_PAD], BF16, tag=f"msg{s}")
                    sl = slice(colbase[s] + off * 8, colbase[s] + (off + ntiles) * 8)
                    nc.gpsimd.dma_gather(
                        t[:, 0:ntiles, :],
                        tabs[s][:],
                        idx_sb[:, sl],
                        ni,
                        ni,
                        OC_PAD,
                        single_packet=(ni <= 1024),
                        queue_num=qctr[0] % CFG["NQ"],
                    )
                    qctr[0] += 1
                    msg_tiles[s][c] = t
                    next_call[s] = c + 1

                def emit_oh():
                    k = next_oh[0]
                    npair = min(CHP, pr.n_pairs - k * CHP)
                    t = oh_pool.tile([128, CHP, 128], BF16, tag="oh")
                    oh_eng = nc.sync if CFG["OH_SYNC"] else nc.gpsimd
                    oh_eng.dma_start(
                        out=t[:, 0:npair, :],
                        in_=oh_d[:, k * CHP : k * CHP + npair, :],
                    )
                    oh_tiles[k] = t
                    next_oh[0] = k + 1

                LOOK = CFG["MSG_BUFS"] - 1

                def mm(pk, pw, start, stop):
                    _, s, t = pr.pairs[pk]
                    while next_call[s] <= min(call_of[s][t] + LOOK, calls[s] - 1):
                        emit_gather(s)
                    while next_oh[0] <= min(pk // CHP + LOOK, n_oh_chunks - 1):
                        emit_oh()
                    oh_ap = oh_tiles[pk // CHP][:, pk % CHP, :]
                    msg_ap = msg_tiles[s][call_of[s][t]][:, slot_of[s][t], 0:OUT_C]
                    nc.tensor.matmul(pw, oh_ap, msg_ap, start=start, stop=stop)

                # PE program order: pass A (stream-0 pairs, windows ascending)
                # then pass B (stream-1 + evac). One accumulation group per
                # PSUM bank (start zeroes the whole 2 KB zero-region): flag
                # start on the bank's first matmul, stop on its last.
                first_mm, last_mm = {}, {}
                order = [pk for w in range(WIN) for pk in pr.segs[w][0]] + [
                    pk for w in range(WIN) for pk in pr.segs[w][1]
                ]
                for pk in order:
                    b = pr.pairs[pk][0] // 8
                    first_mm.setdefault(b, pk)
                    last_mm[b] = pk

                # pass A: stream-0 pairs of every window, psums stay resident
                # (8 windows per PSUM bank, all 64 windows across the 8 banks)
                pw_banks = {}
                for w in range(WIN):
                    segA, _ = pr.segs[w]
                    if w % 8 == 0:
                        pw_banks[w // 8] = psum_pool.tile(
                            [128, 8, OUT_C], F32, tag="w", name="pwb"
                        )
                    pw = pw_banks[w // 8][:, w % 8, :]
                    b = w // 8
                    for pk in segA:
                        mm(pk, pw, pk == first_mm[b], pk == last_mm[b])
                    # advance stream-1 gathers proportionally during pass A so
                    # pass B (and the next AllGather it feeds) starts sooner;
                    # offset start so AG-b has landed before B0 dispatches
                    bt = max(0, ((w - 6) * calls[1]) // (WIN - 6))
                    while next_call[1] < min(bt, calls[1]):
                        emit_gather(1)
                # pass B: stream-1 pairs, then evacuate
                for w in range(WIN):
                    _, segB = pr.segs[w]
                    pw = pw_banks[w // 8][:, w % 8, :]
                    b = w // 8
                    for pk in segB:
                        mm(pk, pw, pk == first_mm[b], pk == last_mm[b])
                    evac(w, pw)
                    if w == WH - 1 and mid_cb:
                        mid_cb()
                    elif w == WIN - 1 and end_cb:
                        end_cb()

            # ---- hop 1:  z1 = (psum + z0) / deg ----
            def evac1(w, pw):
                tmp = tmp_pool.tile([128, OUT_C], F32, tag="tmp")
                nc.vector.tensor_add(tmp[:], pw, zpad1[:, w, 0:OUT_C])
                if CFG["ACT_EVAC"]:
                    nc.scalar.mul(
                        zpad2[:, w, 0:OUT_C], tmp[:], deginv[:, w : w + 1]
                    )
                else:
                    nc.vector.tensor_scalar(
                        zpad2[:, w, 0:OUT_C], tmp[:], deginv[:, w : w + 1], None, Alu.mult
                    )

            run_hop(
                cc_out[0],
                evac1,
                mid_cb=lambda: emit_ag(1, 0, zpad2),
                end_cb=lambda: emit_ag(1, 1, zpad2),
            )

            # ---- hop 2:  out = dinv * (psum + z1) + b ----
            def evac2(w, pw):
                tmp = tmp_pool.tile([128, OUT_C], F32, tag="tmp")
                tmp2 = tmp_pool.tile([128, OUT_C], F32, tag="tmp2")
                nc.vector.tensor_add(tmp[:], pw, zpad2[:, w, 0:OUT_C])
                if CFG["ACT_EVAC"]:
                    nc.scalar.mul(tmp2[:], tmp[:], dinv[:, w : w + 1])
                else:
                    nc.vector.tensor_scalar(
                        tmp2[:], tmp[:], dinv[:, w : w + 1], None, Alu.mult
                    )
                nc.vector.tensor_add(outst[:, w, :], tmp2[:], b_sb[:])

            run_hop(cc_out[1], evac2)
            nc.sync.dma_start(out_d[:], outst[:])

    nc.compile()
    return nc


def _make_in_maps(pr, x, W, b):
    C, NP, WIN = pr.C, pr.NP, pr.WIN
    x = np.asarray(x, dtype=np.float32)
    W = np.asarray(W, dtype=np.float32)
    b = np.asarray(b, dtype=np.float32)
    wt = np.ascontiguousarray(W.T)
    b_rep = np.ascontiguousarray(np.broadcast_to(b, (128, len(b))))
    in_maps = []
    for i in range(C):
        xt = np.ascontiguousarray(x[i * NP : (i + 1) * NP].T)
        in_maps.append(
            dict(
                xt=xt,
                wt=wt,
                bias=b_rep,
                deg=pr.deg_staged[i],
                idx=pr.idx_wrapped[i],
                oh=pr.onehot[i],
            )
        )
    return in_maps


def _unpermute(o, pr):
    # device rows are p*WIN+w; node order is w*128+p
    return (
        o.reshape(128, pr.WIN, o.shape[-1])
        .transpose(1, 0, 2)
        .reshape(pr.NP, o.shape[-1])
    )


_CACHE = {}


def kernel(x, edge_index, W, b):
    pr = _preprocess(edge_index)
    nc = _build(pr)
    in_maps = _make_in_maps(pr, x, W, b)

    from concourse import bass_utils

    res = bass_utils.run_bass_kernel_spmd(
        nc, in_maps, core_ids=list(range(pr.C))
    )
    shards = [_unpermute(res.results[i]["out"], pr) for i in range(pr.C)]
    return np.ascontiguousarray(np.concatenate(shards, axis=0))

